# revision 1
# baseline (speedup 1.0000x reference)
"""Trainium2 Bass kernel for bipartite GNN metapath aggregation (LATTE).

Reference math:
    h_a = relu(x_a @ W_a + b_a); h_b = relu(x_b @ W_b + b_b)
    A[r,c] = #edges_ab(r,c); B[r,c] = #edges_ba(r,c)
    deg = colcount(A) + rowcount(B); d = 1/deg (0 where deg==0)
    out = (A*d) @ B @ h_a + A @ h_b
Reassociated (exact up to fp reassociation):
    out = A @ (d[:,None] * (B @ h_a) + h_b)

Distribution: 1D row-parallel over 8 NeuronCores; 512 rows per core.
Device does: projections (fp32 matmul), SpMM as dense bf16 matmuls with
exact small-int adjacency entries and hi/lo-split bf16 activations
(recovers ~fp32 accuracy), two AllGathers (h_a, v).
"""

import numpy as np
import ml_dtypes

NA = 4096
NB = 4096
FA = 512
FB = 512
D = 128
M = 8  # cores
S = NA // M  # 512 rows per core
KT = NA // 128  # 32 k-tiles over the node dim
P = 128

_BUILT = {}


def _emit(nc, tc, tensors, split_ha=True, split_v=True):
    import concourse.mybir as mybir
    from concourse.masks import make_identity

    f32 = mybir.dt.float32
    bf16 = mybir.dt.bfloat16
    Relu = mybir.ActivationFunctionType.Relu
    Copy = mybir.ActivationFunctionType.Copy
    SUB = mybir.AluOpType.subtract

    (xaT, xbT, Wa, Wb, ba_rep, bb_rep, ATs, BTs, dsw, outT) = tensors

    ctxs = []

    def pool(name, bufs, space="SBUF"):
        p = tc.tile_pool(name=name, bufs=bufs, space=space)
        ctxs.append(p)
        return p.__enter__()

    constp = pool("const", 1)
    bigp = pool("big", 1)
    workp = pool("work", 3)
    psp = pool("ps", 2, "PSUM")
    accp = pool("acc", 2, "PSUM")
    trp = pool("tr", 2, "PSUM")
    dramp = pool("dram", 1, "DRAM")

    try:
        ident = constp.tile([P, P], f32, tag="ident")
        make_identity(nc, ident[:])

        # ---- resident loads -------------------------------------------------
        xa_sb = bigp.tile([P, 4, S], f32, tag="xa")
        xb_sb = bigp.tile([P, 4, S], f32, tag="xb")
        wa_sb = constp.tile([P, 4, D], f32, tag="wa")
        wb_sb = constp.tile([P, 4, D], f32, tag="wb")
        ba_sb = constp.tile([P, P], f32, tag="ba")
        bb_sb = constp.tile([P, P], f32, tag="bb")
        d_sb = constp.tile([P, 4], f32, tag="d")
        at_sb = bigp.tile([P, KT, S], bf16, tag="at")
        bt_sb = bigp.tile([P, KT, S], bf16, tag="bt")

        # xaT gates the first projection (critical path to AllGather#1):
        # load per k-tile so matmul k=0 starts after the first 256KB.
        for k in range(4):
            nc.sync.dma_start(out=xa_sb[:, k, :], in_=xaT[:, k, :])
        nc.sync.dma_start(out=xb_sb[:], in_=xbT[:])
        nc.sync.dma_start(out=wa_sb[:], in_=Wa[:])
        nc.sync.dma_start(out=wb_sb[:], in_=Wb[:])
        nc.sync.dma_start(out=ba_sb[:], in_=ba_rep[:])
        nc.sync.dma_start(out=bb_sb[:], in_=bb_rep[:])
        nc.sync.dma_start(out=d_sb[:], in_=dsw[:])
        nc.sync.dma_start(out=bt_sb[:], in_=BTs[:])
        nc.sync.dma_start(out=at_sb[:], in_=ATs[:])

        HCa = 2 * D if split_ha else D
        HCv = 2 * D if split_v else D
        hdt = bf16

        def hilo(src_f32, dst, doff, split):
            """dst[:, doff:doff+2D or D] <- hi(||lo) bf16 split of src_f32 [P, D]."""
            nc.vector.tensor_copy(dst[:, doff : doff + D], src_f32)
            if not split:
                return
            hi32 = workp.tile([P, D], f32, tag="hi32")
            nc.vector.tensor_copy(hi32[:], dst[:, doff : doff + D])
            lo32 = workp.tile([P, D], f32, tag="lo32")
            nc.vector.tensor_tensor(out=lo32[:], in0=src_f32, in1=hi32[:], op=SUB)
            nc.vector.tensor_copy(dst[:, doff + D : doff + 2 * D], lo32[:])

        def proj(x_sb, w_sb, b_sb):
            """returns 4 node-major fp32 SBUF tiles [128 rows, 128 D]."""
            outs = []
            for ri in range(4):
                ps = psp.tile([P, D], f32, tag="proj")
                for k in range(4):
                    nc.tensor.matmul(
                        ps[:],
                        lhsT=x_sb[:, k, ri * P : (ri + 1) * P],
                        rhs=w_sb[:, k, :],
                        start=(k == 0),
                        stop=(k == 3),
                    )
                hsum = workp.tile([P, D], f32, tag="hsum")
                nc.vector.tensor_add(out=hsum[:], in0=ps[:], in1=b_sb[:])
                hf = workp.tile([P, D], f32, tag="hf")
                nc.scalar.activation(hf[:], hsum[:], Relu)
                outs.append(hf)
            return outs

        # ---- h_a: project, split, allgather --------------------------------
        ag_ha_in = dramp.tile([S, HCa], hdt, tag="aghi")
        ag_ha_out = dramp.tile([NA, HCa], hdt, tag="agho", addr_space="Shared")

        ha_tiles = proj(xa_sb, wa_sb, ba_sb)
        for ri in range(4):
            hl = workp.tile([P, HCa], hdt, tag="hl")
            hilo(ha_tiles[ri][:], hl, 0, split_ha)
            nc.sync.dma_start(
                out=ag_ha_in[ri * P : (ri + 1) * P, :], in_=hl[:]
            )
        nc.gpsimd.collective_compute(
            "AllGather",
            mybir.AluOpType.bypass,
            replica_groups=[list(range(M))],
            ins=[ag_ha_in[:].opt()],
            outs=[ag_ha_out[:].opt()],
        )
        ha_sb = bigp.tile([P, KT, HCa], hdt, tag="haf")
        _hav = ag_ha_out[:].rearrange("(k p) c -> p k c", p=P)
        for ci in range(4):
            ksl = slice(ci * KT // 4, (ci + 1) * KT // 4)
            nc.sync.dma_start(out=ha_sb[:, ksl, :], in_=_hav[:, ksl, :])

        # ---- h_b local (node-major fp32) -----------------------------------
        hb_tiles = proj(xb_sb, wb_sb, bb_sb)

        # ---- SpMM1: tT = (B_shard @ h_a)^T  [D, S] -------------------------
        tps = accp.tile([P, S], f32, tag="acc")
        for k in range(KT):
            nc.tensor.matmul(
                tps[:],
                lhsT=ha_sb[:, k, 0:D],
                rhs=bt_sb[:, k, :],
                start=(k == 0),
                stop=(not split_ha and k == KT - 1),
            )
            if split_ha:
                nc.tensor.matmul(
                    tps[:],
                    lhsT=ha_sb[:, k, D : 2 * D],
                    rhs=bt_sb[:, k, :],
                    start=False,
                    stop=(k == KT - 1),
                )
        tT_sb = workp.tile([P, S], f32, tag="tT")
        nc.vector.tensor_copy(tT_sb[:], tps[:])

        # ---- v = d*t + h_b, node-major, split, allgather -------------------
        ag_v_in = dramp.tile([S, HCv], hdt, tag="agvi")
        ag_v_out = dramp.tile([NA, HCv], hdt, tag="agvo", addr_space="Shared")
        for ri in range(4):
            trps = trp.tile([P, P], f32, tag="tr")
            nc.tensor.transpose(
                out=trps[:], in_=tT_sb[:, ri * P : (ri + 1) * P], identity=ident[:]
            )
            v32 = workp.tile([P, D], f32, tag="v32")
            nc.scalar.activation(
                v32[:], trps[:], Copy, scale=d_sb[:, ri : ri + 1]
            )
            nc.vector.tensor_add(out=v32[:], in0=v32[:], in1=hb_tiles[ri][:])
            vl = workp.tile([P, HCv], hdt, tag="vl")
            hilo(v32[:], vl, 0, split_v)
            nc.sync.dma_start(out=ag_v_in[ri * P : (ri + 1) * P, :], in_=vl[:])
        nc.gpsimd.collective_compute(
            "AllGather",
            mybir.AluOpType.bypass,
            replica_groups=[list(range(M))],
            ins=[ag_v_in[:].opt()],
            outs=[ag_v_out[:].opt()],
        )
        v_sb = bigp.tile([P, KT, HCv], hdt, tag="vf")
        _vv = ag_v_out[:].rearrange("(k p) c -> p k c", p=P)
        for ci in range(4):
            ksl = slice(ci * KT // 4, (ci + 1) * KT // 4)
            nc.sync.dma_start(out=v_sb[:, ksl, :], in_=_vv[:, ksl, :])

        # ---- SpMM2: outT = (A_shard @ v)^T  [D, S] -------------------------
        ops = accp.tile([P, S], f32, tag="acc")
        for k in range(KT):
            nc.tensor.matmul(
                ops[:],
                lhsT=v_sb[:, k, 0:D],
                rhs=at_sb[:, k, :],
                start=(k == 0),
                stop=(not split_v and k == KT - 1),
            )
            if split_v:
                nc.tensor.matmul(
                    ops[:],
                    lhsT=v_sb[:, k, D : 2 * D],
                    rhs=at_sb[:, k, :],
                    start=False,
                    stop=(k == KT - 1),
                )
        o_sb = workp.tile([P, S], f32, tag="osb")
        nc.vector.tensor_copy(o_sb[:], ops[:])
        nc.sync.dma_start(out=outT[:], in_=o_sb[:])
    finally:
        for p in reversed(ctxs):
            p.__exit__(None, None, None)


def _build(split=True):
    split_ha, split_v = split if isinstance(split, tuple) else (split, split)
    key = ("nc", split_ha, split_v)
    if key in _BUILT:
        return _BUILT[key]
    import concourse.bacc as bacc
    import concourse.mybir as mybir
    import concourse.tile as tile

    f32 = mybir.dt.float32
    bf16 = mybir.dt.bfloat16

    nc = bacc.Bacc("TRN2", target_bir_lowering=False, debug=False, num_devices=M)
    xaT = nc.declare_dram_parameter("xaT", [P, 4, S], f32, isOutput=False)
    xbT = nc.declare_dram_parameter("xbT", [P, 4, S], f32, isOutput=False)
    Wa = nc.declare_dram_parameter("Wa", [P, 4, D], f32, isOutput=False)
    Wb = nc.declare_dram_parameter("Wb", [P, 4, D], f32, isOutput=False)
    ba = nc.declare_dram_parameter("ba", [P, P], f32, isOutput=False)
    bb = nc.declare_dram_parameter("bb", [P, P], f32, isOutput=False)
    ATs = nc.declare_dram_parameter("ATs", [P, KT, S], bf16, isOutput=False)
    BTs = nc.declare_dram_parameter("BTs", [P, KT, S], bf16, isOutput=False)
    dsw = nc.declare_dram_parameter("dsw", [P, 4], f32, isOutput=False)
    outT = nc.declare_dram_parameter("outT", [P, S], f32, isOutput=True)

    with tile.TileContext(nc) as tc:
        _emit(
            nc,
            tc,
            (
                xaT[:],
                xbT[:],
                Wa[:],
                Wb[:],
                ba[:],
                bb[:],
                ATs[:],
                BTs[:],
                dsw[:],
                outT[:],
            ),
            split_ha=split_ha,
            split_v=split_v,
        )
    nc.compile()
    _BUILT[key] = nc
    return nc


def _swz(a, kt):
    """[kt*128, n] row-major -> [128, kt, n] partition-major contiguous."""
    n = a.shape[1]
    return np.ascontiguousarray(a.reshape(kt, P, n).transpose(1, 0, 2))


def make_in_maps(
    x_a, x_b, W_a, b_a, W_b, b_b, edge_index_ab, edge_index_ba, split=True
):
    bf = ml_dtypes.bfloat16
    x_a = np.asarray(x_a, np.float32)
    x_b = np.asarray(x_b, np.float32)
    W_a = np.asarray(W_a, np.float32)
    W_b = np.asarray(W_b, np.float32)
    b_a = np.asarray(b_a, np.float32).reshape(-1)
    b_b = np.asarray(b_b, np.float32).reshape(-1)
    ea = np.asarray(edge_index_ab).astype(np.int64)
    eb = np.asarray(edge_index_ba).astype(np.int64)

    # Dense transposed adjacencies with duplicate accumulation.
    AT = (
        np.bincount(ea[1] * NA + ea[0], minlength=NA * NB)
        .reshape(NB, NA)
        .astype(np.float32)
    )  # AT[c, r] = A[r, c]
    BT = (
        np.bincount(eb[1] * NB + eb[0], minlength=NA * NB)
        .reshape(NA, NB)
        .astype(np.float32)
    )  # BT[c, r] = B[r, c]
    deg = (
        np.bincount(ea[1], minlength=NB) + np.bincount(eb[0], minlength=NB)
    ).astype(np.float32)
    d = np.where(
        deg > 0, np.float32(1.0) / np.maximum(deg, np.float32(1.0)), np.float32(0.0)
    ).astype(np.float32)

    xaT_f = np.ascontiguousarray(x_a.T)  # [FA, NA]
    xbT_f = np.ascontiguousarray(x_b.T)
    AT_bf = AT.astype(bf)
    BT_bf = BT.astype(bf)
    wa_sw = _swz(W_a, 4)
    wb_sw = _swz(W_b, 4)
    ba_rep = np.ascontiguousarray(np.broadcast_to(b_a, (P, P))).astype(np.float32)
    bb_rep = np.ascontiguousarray(np.broadcast_to(b_b, (P, P))).astype(np.float32)

    in_maps = []
    for m in range(M):
        sl = slice(m * S, (m + 1) * S)
        in_maps.append(
            {
                "xaT": _swz(xaT_f[:, sl], 4),
                "xbT": _swz(xbT_f[:, sl], 4),
                "Wa": wa_sw,
                "Wb": wb_sw,
                "ba": ba_rep,
                "bb": bb_rep,
                "ATs": _swz(np.ascontiguousarray(AT_bf[:, sl]), KT),
                "BTs": _swz(np.ascontiguousarray(BT_bf[:, sl]), KT),
                "dsw": np.ascontiguousarray(d[sl].reshape(4, P).T),
            }
        )
    return in_maps


def run(inputs, split=True, trace=False):
    from concourse.bass_utils import run_bass_kernel_spmd

    nc = _build(split=split)
    in_maps = make_in_maps(**inputs, split=split)
    res = run_bass_kernel_spmd(nc, in_maps, core_ids=list(range(M)), trace=trace)
    out = np.concatenate([np.asarray(r["outT"]).T for r in res.results], axis=0)
    return out.astype(np.float32), res


def kernel(**inputs):
    # h_a plain bf16 (its error is attenuated ~deg× by the d-normalization
    # before reaching the output); v keeps the hi/lo split. HW-measured
    # rel err 6.8e-5 vs 1.5e-6 for full split, ~15% faster end-to-end.
    out, _ = run(inputs, split=(False, True), trace=False)
    return out



# revision 15
# speedup vs baseline: 634.9414x; 634.9414x over previous
"""Trainium2 Bass kernel for bipartite GNN metapath aggregation (LATTE).

Reference math:
    h_a = relu(x_a @ W_a + b_a); h_b = relu(x_b @ W_b + b_b)
    A[r,c] = #edges_ab(r,c); B[r,c] = #edges_ba(r,c)
    deg = colcount(A) + rowcount(B); d = 1/deg (0 where deg==0)
    out = (A*d) @ B @ h_a + A @ h_b
Reassociated (exact up to fp reassociation):
    out = A @ ((d[:,None]*B) @ h_a + h_b)

Distribution: 1D row-parallel over 8 NeuronCores; 512 output rows per
core. All matmul operands bf16 (fp32 PSUM accumulation); d folded into
B on the host; measured rel err ~1.3e-3 (budget 2e-2).

The h_a projection is REPLICATED: every core projects the full x_a
(4 MB replicated load) instead of gathering projected shards. That
removes the first AllGather entirely, so SpMM1 runs as soon as the
local projection + B load finish — none of it waits on the runtime's
NEFF-entry collective barrier (~25 us skew absorb + ~15 us ncfw
latency), which instead overlaps local compute. The single remaining
collective (v = d*(B@h_a)+h_b, bf16, 1 MB gathered) is split into four
quarter-shard AllGathers so SpMM2 consumes quarter k-ranges while
later quarters are still on the wire; the host permutes A's k-tile
order to match. Filler matmuls on resident tiles keep the PE clock
warm across the collective-wait window (cold-PE matmuls measure
~630 ns vs ~270 ns warm for the same 512-col bf16 shape).
"""

import numpy as np
import ml_dtypes

NA = 4096
NB = 4096
FA = 512
FB = 512
D = 128
M = 8  # cores
S = NA // M  # 512 output rows per core
SPLITS = (3, 1)  # v AllGather split sizes, in 128-row blocks (sum 4)
KT = NA // 128  # 32 k-tiles over the node dim
P = 128
N_FILL = 48  # PE-warming filler matmuls during the collective wait

_BUILT = {}


def _emit(nc, tc, t):
    import concourse.mybir as mybir
    from concourse.masks import make_identity

    f32 = mybir.dt.float32
    bf16 = mybir.dt.bfloat16
    Relu = mybir.ActivationFunctionType.Relu

    ctxs = []

    def pool(name, bufs, space="SBUF"):
        p = tc.tile_pool(name=name, bufs=bufs, space=space)
        ctxs.append(p)
        return p.__enter__()

    constp = pool("const", 1)
    bigp = pool("big", 1)
    workp = pool("work", 2)
    psp = pool("ps", 2, "PSUM")
    accp = pool("acc", 1, "PSUM")
    trp = pool("tr", 2, "PSUM")
    dramp = pool("dram", 1, "DRAM")

    def allgather(src, dst):
        nc.gpsimd.collective_compute(
            "AllGather",
            mybir.AluOpType.bypass,
            replica_groups=[list(range(M))],
            ins=[src.opt()],
            outs=[dst.opt()],
        )

    try:
        # ---- dummy collective: its early trigger lets ncfw finish the
        # NEFF-entry barrier + arming while the compute front runs, so the
        # first real gather starts promptly once v is ready ----------------
        dumb_in = dramp.tile([1, 16], bf16, tag="dumbi")
        dumb_out = dramp.tile([M, 16], bf16, tag="dumbo", addr_space="Shared")
        allgather(dumb_in[:], dumb_out[:])

        # ---- input DMA, in priority order ----------------------------------
        wa_sb = constp.tile([P, 4, D], bf16, tag="wa")
        bap_sb = constp.tile([P, 1], f32, tag="bap")
        nc.sync.dma_start(out=wa_sb[:], in_=t["Wa"])
        nc.sync.dma_start(out=bap_sb[:], in_=t["bap"])

        # full (replicated) x_a^T and the B shard gate SpMM1: interleaved
        # eighth-slices so compute waits only on the slice it reads.
        xa_q, bt_q = [], []
        for c in range(8):
            xq = bigp.tile([P, 4, NA // 8], bf16, name=f"xa{c}", tag=f"xa{c}")
            bq = bigp.tile(
                [P, KT // 8, S], bf16, name=f"bt{c}", tag=f"bt{c}"
            )
            xa_q.append(xq)
            bt_q.append(bq)
            nsl = slice(c * NA // 8, (c + 1) * NA // 8)
            nc.sync.dma_start(out=xq[:], in_=t["xaT"][:, :, nsl])
            ksl = slice(c * KT // 8, (c + 1) * KT // 8)
            nc.sync.dma_start(out=bq[:], in_=t["BTs"][:, ksl, :])

        xb_sb = bigp.tile([P, 4, S], bf16, tag="xb")
        wb_sb = constp.tile([P, 4, D], bf16, tag="wb")
        bbp_sb = constp.tile([P, 1], f32, tag="bbp")
        nc.sync.dma_start(out=xb_sb[:], in_=t["xbT"])
        nc.sync.dma_start(out=wb_sb[:], in_=t["Wb"])
        nc.sync.dma_start(out=bbp_sb[:], in_=t["bbp"])

        at_q = []
        for c in range(4):
            aq = bigp.tile([P, KT // 4, S], bf16, name=f"at{c}", tag=f"at{c}")
            at_q.append(aq)
            ksl = slice(c * KT // 4, (c + 1) * KT // 4)
            nc.sync.dma_start(out=aq[:], in_=t["ATs"][:, ksl, :])

        ident = constp.tile([P, P], bf16, tag="ident")
        make_identity(nc, ident[:])

        # ---- replicated projection: h_a^T = relu(W_a^T @ x_a^T + b_a) ------
        # channel-major wide matmuls, then PE transposes to node-major
        # k-tiles for SpMM1. All local; overlaps the collective barrier.
        haT = bigp.tile([P, NA], bf16, tag="haT")
        ha_nm = [
            bigp.tile([P, P], bf16, name=f"hanm{k}", tag=f"hanm{k}")
            for k in range(KT)
        ]
        for c in range(8):
            nsl = slice(c * 512, (c + 1) * 512)
            ps = psp.tile([P, 512], f32, tag="proj")
            for k in range(4):
                nc.tensor.matmul(
                    ps[:],
                    lhsT=wa_sb[:, k, :],
                    rhs=xa_q[c][:, k, :],
                    start=(k == 0),
                    stop=(k == 3),
                )
            nc.scalar.activation(haT[:, nsl], ps[:], Relu, bias=bap_sb[:, 0:1])
            for j in range(4):
                kt = c * 4 + j
                trps = trp.tile([P, P], bf16, tag="tr")
                nc.tensor.transpose(
                    out=trps[:],
                    in_=haT[:, kt * P : (kt + 1) * P],
                    identity=ident[:],
                )
                nc.vector.tensor_copy(ha_nm[kt][:], trps[:])

        # ---- h_b^T: [D partitions, S nodes] --------------------------------
        hps = accp.tile([P, S], f32, tag="hbps")
        for k in range(4):
            nc.tensor.matmul(
                hps[:],
                lhsT=wb_sb[:, k, :],
                rhs=xb_sb[:, k, :],
                start=(k == 0),
                stop=(k == 3),
            )
        hbT = workp.tile([P, S], f32, tag="hbT")
        nc.scalar.activation(hbT[:], hps[:], Relu, bias=bbp_sb[:, 0:1])

        # ---- SpMM1: v^T = (dB)_shard @ h_a + h_b^T -------------------------
        tps = accp.tile([P, S], f32, tag="acc")
        for k in range(KT):
            nc.tensor.matmul(
                tps[:],
                lhsT=ha_nm[k][:],
                rhs=bt_q[k // 4][:, k % 4, :],
                start=(k == 0),
                stop=(k == KT - 1),
            )
        vT = workp.tile([P, S], bf16, tag="vT")
        nc.vector.tensor_add(out=vT[:], in0=tps[:], in1=hbT[:])

        # ---- ship v in Q quarter-shard AllGathers --------------------------
        bounds = [0]
        for w in SPLITS:
            bounds.append(bounds[-1] + w * P)
        ag_v_in = dramp.tile([S, D], bf16, tag="agvi")
        ag_v_out = [
            dramp.tile(
                [M * (bounds[h + 1] - bounds[h]), D], bf16,
                name=f"agvo{h}", tag=f"agvo{h}", addr_space="Shared",
            )
            for h in range(len(SPLITS))
        ]
        h = 0
        for ri in range(4):
            trps = trp.tile([P, P], bf16, tag="tr")
            nc.tensor.transpose(
                out=trps[:], in_=vT[:, ri * P : (ri + 1) * P], identity=ident[:]
            )
            vl = workp.tile([P, D], bf16, tag="vl")
            nc.vector.tensor_copy(vl[:], trps[:])
            nc.sync.dma_start(out=ag_v_in[ri * P : (ri + 1) * P, :], in_=vl[:])
            if (ri + 1) * P == bounds[h + 1]:
                allgather(
                    ag_v_in[bounds[h] : bounds[h + 1], :], ag_v_out[h][:]
                )
                h += 1

        # ---- PE-warming fillers during the collective wait -----------------
        fps = psp.tile([P, 512], f32, tag="proj")
        for i in range(N_FILL):
            nc.tensor.matmul(
                fps[:],
                lhsT=ha_nm[i % KT][:],
                rhs=bt_q[i % 8][:, i % 4, :],
                start=True,
                stop=True,
                skip_group_check=True,
            )

        # ---- SpMM2: out^T += v^T sub-gathers @ A-kt ------------------------
        ops = accp.tile([P, S], f32, tag="acc")
        k = 0
        for h, w in enumerate(SPLITS):
            kq = w * M  # k-tiles delivered by this sub-gather
            src = ag_v_out[h][:].rearrange("(k p) c -> p k c", p=P)
            vc = bigp.tile([P, kq, D], bf16, name=f"v{h}", tag=f"v{h}")
            nc.sync.dma_start(out=vc[:], in_=src[:])
            for j in range(kq):
                nc.tensor.matmul(
                    ops[:],
                    lhsT=vc[:, j, :],
                    rhs=at_q[k // 8][:, k % 8, :],
                    start=(k == 0),
                    stop=(k == KT - 1),
                )
                k += 1
        o_sb = workp.tile([P, S], f32, tag="osb")
        nc.vector.tensor_copy(o_sb[:], ops[:])
        nc.sync.dma_start(out=t["outT"], in_=o_sb[:])
    finally:
        for p in reversed(ctxs):
            p.__exit__(None, None, None)


def _build():
    if "nc" in _BUILT:
        return _BUILT["nc"]
    import concourse.bacc as bacc
    import concourse.mybir as mybir
    import concourse.tile as tile

    f32 = mybir.dt.float32
    bf16 = mybir.dt.bfloat16

    nc = bacc.Bacc("TRN2", target_bir_lowering=False, debug=False, num_devices=M)
    decl = nc.declare_dram_parameter
    t = {
        "xaT": decl("xaT", [P, 4, NA], bf16, isOutput=False)[:],
        "xbT": decl("xbT", [P, 4, S], bf16, isOutput=False)[:],
        "Wa": decl("Wa", [P, 4, D], bf16, isOutput=False)[:],
        "Wb": decl("Wb", [P, 4, D], bf16, isOutput=False)[:],
        "bap": decl("bap", [P, 1], f32, isOutput=False)[:],
        "bbp": decl("bbp", [P, 1], f32, isOutput=False)[:],
        "ATs": decl("ATs", [P, KT, S], bf16, isOutput=False)[:],
        "BTs": decl("BTs", [P, KT, S], bf16, isOutput=False)[:],
        "outT": decl("outT", [P, S], f32, isOutput=True)[:],
    }

    with tile.TileContext(nc) as tc:
        _emit(nc, tc, t)
    nc.compile()
    _BUILT["nc"] = nc
    return nc


def _swz(a, kt):
    """[kt*128, n] row-major -> [128, kt, n] partition-major contiguous."""
    n = a.shape[1]
    return np.ascontiguousarray(a.reshape(kt, P, n).transpose(1, 0, 2))


def _gather_perm():
    """Node order produced by the sub-shard AllGathers of v.

    Sub-gather h concatenates rows [lo, lo+w*P) of every core's S-row
    shard, so gathered row r of sub-gather h is node
    (r // (w*P)) * S + lo + (r % (w*P)).
    """
    parts = []
    lo = 0
    for w in SPLITS:
        n = w * P
        r = np.arange(M * n)
        parts.append((r // n) * S + lo + (r % n))
        lo += n
    return np.concatenate(parts)


def make_in_maps(x_a, x_b, W_a, b_a, W_b, b_b, edge_index_ab, edge_index_ba):
    bf = ml_dtypes.bfloat16
    x_a = np.asarray(x_a, np.float32)
    x_b = np.asarray(x_b, np.float32)
    W_a = np.asarray(W_a, np.float32)
    W_b = np.asarray(W_b, np.float32)
    b_a = np.asarray(b_a, np.float32).reshape(-1)
    b_b = np.asarray(b_b, np.float32).reshape(-1)
    ea = np.asarray(edge_index_ab).astype(np.int64)
    eb = np.asarray(edge_index_ba).astype(np.int64)

    # Dense transposed adjacencies with duplicate accumulation.
    AT = (
        np.bincount(ea[1] * NA + ea[0], minlength=NA * NB)
        .reshape(NB, NA)
        .astype(np.float32)
    )  # AT[c, r] = A[r, c]
    BT = (
        np.bincount(eb[1] * NB + eb[0], minlength=NA * NB)
        .reshape(NA, NB)
        .astype(np.float32)
    )  # BT[c, r] = B[r, c]
    deg = (
        np.bincount(ea[1], minlength=NB) + np.bincount(eb[0], minlength=NB)
    ).astype(np.float32)
    d = np.where(
        deg > 0, np.float32(1.0) / np.maximum(deg, np.float32(1.0)), np.float32(0.0)
    ).astype(np.float32)
    BTd = (BT * d[None, :]).astype(bf)  # d folded into B; natural row order
    AT_bf = AT.astype(bf)[_gather_perm()]  # rows in v-gather order

    xaT_sw = _swz(np.ascontiguousarray(x_a.T).astype(bf), 4)  # replicated
    xbT_f = np.ascontiguousarray(x_b.T).astype(bf)
    wa_sw = _swz(W_a, 4).astype(bf)
    wb_sw = _swz(W_b, 4).astype(bf)
    bap = np.ascontiguousarray(b_a.reshape(P, 1)).astype(np.float32)
    bbp = np.ascontiguousarray(b_b.reshape(P, 1)).astype(np.float32)

    in_maps = []
    for m in range(M):
        sl = slice(m * S, (m + 1) * S)
        in_maps.append(
            {
                "xaT": xaT_sw,
                "xbT": _swz(xbT_f[:, sl], 4),
                "Wa": wa_sw,
                "Wb": wb_sw,
                "bap": bap,
                "bbp": bbp,
                "ATs": _swz(np.ascontiguousarray(AT_bf[:, sl]), KT),
                "BTs": _swz(np.ascontiguousarray(BTd[:, sl]), KT),
            }
        )
    return in_maps


def run(inputs, trace=False, **trace_kwargs):
    from concourse.bass_utils import run_bass_kernel_spmd

    nc = _build()
    in_maps = make_in_maps(**inputs)
    res = run_bass_kernel_spmd(
        nc, in_maps, core_ids=list(range(M)), trace=trace, **trace_kwargs
    )
    out = np.concatenate([np.asarray(r["outT"]).T for r in res.results], axis=0)
    return out.astype(np.float32), res


def kernel(**inputs):
    out, _ = run(inputs, trace=False)
    return out
